# revision 10
# baseline (speedup 1.0000x reference)
"""Trainium2 Bass kernel for nn_DenseLocal: out = softplus(einsum('bki,kio->bko', x, kernels)).

Shapes (hardcoded): x [512, 128, 1024] f32, kernels [128, 1024, 1024] f32,
out [512, 128, 1024] f32.

Strategy: shard the 128 position-kernels across 8 NeuronCores (16 each,
expert-style).  Per core, each position k is an independent [512,1024] @
[1024,1024] GEMM followed by softplus.  Inputs are cast to bf16 on the host
(fp32 matmul is 4x slower on the PE; bf16 accumulates in fp32 PSUM), x is
pre-transposed on the host so the contraction dim lands on SBUF partitions.
Softplus is computed as Ln(Exp(z) + 1) on the ScalarE — both functions live
in one LUT table set; Softplus itself is not in this compiler's act tables.
"""

import sys
import types

import ml_dtypes
import numpy as np

BF16 = ml_dtypes.bfloat16

B = 512          # batch
K = 128          # n_kernels (position axis)
I = 1024         # in_dim
U = 1024         # units
NCORES = 8
RK = K // NCORES  # kernels per core
P = 128           # SBUF partitions
IC = I // P       # 8 contraction chunks
NCK = U // 512    # 2 moving chunks per units dim


def _ensure_axon_hooks():
    """The image's antenv package lacks axon_hooks; inject a minimal registry
    so run_bass_kernel_spmd(trace=True) can find the NTFF profile hook."""
    if "antenv.axon_hooks" in sys.modules:
        return
    hooks = types.ModuleType("antenv.axon_hooks")
    hooks._hook = None

    def _set(h):
        hooks._hook = h

    def _get():
        return hooks._hook

    hooks.set_axon_ntff_profile_hook = _set
    hooks.get_axon_ntff_profile_hook = _get
    try:
        import antenv

        sys.modules["antenv.axon_hooks"] = hooks
        antenv.axon_hooks = hooks
    except ImportError:
        pass


_ensure_axon_hooks()

import concourse.mybir as mybir  # noqa: E402
import concourse.tile as tile  # noqa: E402
from concourse import bacc  # noqa: E402
from concourse.bass_utils import run_bass_kernel_spmd  # noqa: E402
from concourse.hw_specs import get_activation_tables  # noqa: E402


def _dedupe_act_table_loads(nc):
    """bacc's insert_act_table_loads alternates exp_and_others /
    natural_log per activation (64 reloads x ~1.3us).  Both Exp and Ln
    live in the single natural_log_exp_and_others set: retarget the first
    load to it and drop the rest."""
    set_id = list(get_activation_tables(nc.m.arch)).index(
        "natural_log_exp_and_others"
    )
    first = True
    for blk in nc.main_func.blocks:
        drop = []
        for idx, inst in enumerate(blk.instructions):
            if isinstance(inst, mybir.InstLoadActFuncSet):
                assert inst.sync_info is None or (
                    not inst.sync_info.on_wait and not inst.sync_info.on_update
                )
                if first:
                    inst.act_func_set_id = set_id
                    first = False
                else:
                    drop.append(idx)
        for idx in reversed(drop):
            del blk.instructions[idx]


def _build():
    """Build the per-core Bass program.

    Per-core DRAM I/O:
      xt [RK, I, B]  bf16 — x shard, transposed per position (contraction-major)
      w  [RK, I, U]  bf16 — kernels shard, natural [in, out] layout
      y  [B, RK, U]  f32  — output shard
    """
    f32 = mybir.dt.float32
    bf16 = mybir.dt.bfloat16

    nc = bacc.Bacc()
    xt = nc.declare_dram_parameter("xt", [RK, I, B], bf16, isOutput=False)
    w = nc.declare_dram_parameter("w", [RK, I, U], bf16, isOutput=False)
    y = nc.declare_dram_parameter("y", [B, RK, U], bf16, isOutput=True)

    with tile.TileContext(nc) as tc:
        with (
            tc.tile_pool(name="xt_pool", bufs=3) as xt_pool,
            tc.tile_pool(name="w_pool", bufs=3) as w_pool,
            tc.tile_pool(name="psum_pool", bufs=2, space="PSUM") as psum_pool,
            tc.tile_pool(name="o_pool", bufs=3) as o_pool,
        ):
            for rk in range(RK):
                # Stage the full [I, B] xT and [I, U] weight slices for this
                # position; contraction dim i = c*128 + p lands on partitions.
                xts = xt_pool.tile([P, IC, B], bf16)
                ws = w_pool.tile([P, IC, U], bf16)
                # Per-contraction-chunk DMAs so the first matmuls can start
                # before the whole 3MB slice has landed.
                for ic in range(IC):
                    nc.sync.dma_start(
                        out=xts[:, ic, :], in_=xt[rk, ic * P : (ic + 1) * P, :]
                    )
                    nc.sync.dma_start(
                        out=ws[:, ic, :], in_=w[rk, ic * P : (ic + 1) * P, :]
                    )

                for bcp in range(2):  # pairs of 128-row batch chunks
                    ps = psum_pool.tile([P, 4, 512], f32)  # 4 PSUM banks
                    for bc_i in range(2):
                        bc = bcp * 2 + bc_i
                        for ic in range(IC):
                            lhsT = xts[:, ic, bc * P : (bc + 1) * P]
                            for nck in range(NCK):
                                nc.tensor.matmul(
                                    ps[:, bc_i * NCK + nck, :],
                                    lhsT,
                                    ws[:, ic, nck * 512 : (nck + 1) * 512],
                                    start=(ic == 0),
                                    stop=(ic == IC - 1),
                                )
                    # softplus(z) = ln(exp(z) + 1); Exp in-place on PSUM,
                    # Ln evicts PSUM -> SBUF.  Both are one LUT table set.
                    nc.scalar.activation(
                        ps[:], ps[:], mybir.ActivationFunctionType.Exp
                    )
                    o = o_pool.tile([P, 4, 512], bf16)
                    nc.scalar.activation(
                        o[:], ps[:], mybir.ActivationFunctionType.Ln, bias=1.0
                    )
                    # Stores ride the SWDGE (GpSimd) so they never stall the
                    # ScalarE activation chain or the input ring; the final
                    # pair goes on the ScalarE HW ring instead, which keeps
                    # the kernel-tail SWDGE quiesce from waiting on them.
                    last = rk == RK - 1 and bcp == 1
                    store_eng = nc.scalar if last else nc.gpsimd
                    for bc_i in range(2):
                        b0 = bcp * 256 + bc_i * P
                        store_eng.dma_start(
                            out=y[b0 : b0 + P, rk].rearrange(
                                "p (c n) -> p c n", c=NCK
                            ),
                            in_=o[:, bc_i * NCK : (bc_i + 1) * NCK, :],
                        )
    nc.compile()
    _dedupe_act_table_loads(nc)
    return nc


_NC_CACHE = None


def _get_nc():
    global _NC_CACHE
    if _NC_CACHE is None:
        _NC_CACHE = _build()
    return _NC_CACHE


def _prep_in_maps(x, kernels):
    in_maps = []
    for c in range(NCORES):
        ks = slice(c * RK, (c + 1) * RK)
        # [B, RK, I] -> [RK, I, B], contraction-major, bf16
        xtc = x[:, ks, :].transpose(1, 2, 0).astype(BF16)
        wc = np.ascontiguousarray(kernels[ks]).astype(BF16)
        in_maps.append({"xt": xtc, "w": wc})
    return in_maps


LAST_RESULT = None  # BassKernelResults of the most recent run (for test harness)


def kernel(x, kernels, _trace=False):
    global LAST_RESULT
    nc = _get_nc()
    in_maps = _prep_in_maps(np.asarray(x), np.asarray(kernels))
    res = run_bass_kernel_spmd(nc, in_maps, list(range(NCORES)), trace=_trace)
    LAST_RESULT = res
    out = np.concatenate([res.results[c]["y"] for c in range(NCORES)], axis=1)
    return out.astype(np.float32)


# revision 12
# speedup vs baseline: 1.0341x; 1.0341x over previous
"""Trainium2 Bass kernel for nn_DenseLocal: out = softplus(einsum('bki,kio->bko', x, kernels)).

Shapes (hardcoded): x [512, 128, 1024] f32, kernels [128, 1024, 1024] f32,
out [512, 128, 1024] f32.

Strategy: shard the 128 position-kernels across 8 NeuronCores (16 each,
expert-style).  Per core, each position k is an independent [512,1024] @
[1024,1024] GEMM followed by softplus.  Inputs are cast to bf16 on the host
(fp32 matmul is 4x slower on the PE; bf16 accumulates in fp32 PSUM), x is
pre-transposed on the host so the contraction dim lands on SBUF partitions.
Softplus is computed as Ln(Exp(z) + 1) on the ScalarE — both functions live
in one LUT table set; Softplus itself is not in this compiler's act tables.
"""

import sys
import types

import ml_dtypes
import numpy as np

BF16 = ml_dtypes.bfloat16

B = 512          # batch
K = 128          # n_kernels (position axis)
I = 1024         # in_dim
U = 1024         # units
NCORES = 8
RK = K // NCORES  # kernels per core
P = 128           # SBUF partitions
IC = I // P       # 8 contraction chunks
NCK = U // 512    # 2 moving chunks per units dim


def _ensure_axon_hooks():
    """The image's antenv package lacks axon_hooks; inject a minimal registry
    so run_bass_kernel_spmd(trace=True) can find the NTFF profile hook."""
    if "antenv.axon_hooks" in sys.modules:
        return
    hooks = types.ModuleType("antenv.axon_hooks")
    hooks._hook = None

    def _set(h):
        hooks._hook = h

    def _get():
        return hooks._hook

    hooks.set_axon_ntff_profile_hook = _set
    hooks.get_axon_ntff_profile_hook = _get
    try:
        import antenv

        sys.modules["antenv.axon_hooks"] = hooks
        antenv.axon_hooks = hooks
    except ImportError:
        pass


_ensure_axon_hooks()

import concourse.mybir as mybir  # noqa: E402
import concourse.tile as tile  # noqa: E402
from concourse import bacc  # noqa: E402
from concourse.bass_utils import run_bass_kernel_spmd  # noqa: E402
from concourse.hw_specs import get_activation_tables  # noqa: E402


def _dedupe_act_table_loads(nc):
    """bacc's insert_act_table_loads alternates exp_and_others /
    natural_log per activation (64 reloads x ~1.3us).  Both Exp and Ln
    live in the single natural_log_exp_and_others set: retarget the first
    load to it and drop the rest."""
    set_id = list(get_activation_tables(nc.m.arch)).index(
        "natural_log_exp_and_others"
    )
    first = True
    for blk in nc.main_func.blocks:
        drop = []
        for idx, inst in enumerate(blk.instructions):
            if isinstance(inst, mybir.InstLoadActFuncSet):
                assert inst.sync_info is None or (
                    not inst.sync_info.on_wait and not inst.sync_info.on_update
                )
                if first:
                    inst.act_func_set_id = set_id
                    first = False
                else:
                    drop.append(idx)
        for idx in reversed(drop):
            del blk.instructions[idx]


def _build():
    """Build the per-core Bass program.

    Per-core DRAM I/O:
      xt [RK, I, B]  bf16 — x shard, transposed per position (contraction-major)
      w  [RK, I, U]  bf16 — kernels shard, natural [in, out] layout
      y  [B, RK, U]  f32  — output shard
    """
    f32 = mybir.dt.float32
    bf16 = mybir.dt.bfloat16

    nc = bacc.Bacc()
    xt = nc.declare_dram_parameter("xt", [RK, I, B], bf16, isOutput=False)
    w = nc.declare_dram_parameter("w", [RK, I, U], bf16, isOutput=False)
    y = nc.declare_dram_parameter("y", [B, RK, U], bf16, isOutput=True)

    with tile.TileContext(nc) as tc:
        with (
            tc.tile_pool(name="xt_pool", bufs=4) as xt_pool,
            tc.tile_pool(name="w_pool", bufs=4) as w_pool,
            tc.tile_pool(name="psum_pool", bufs=2, space="PSUM") as psum_pool,
            tc.tile_pool(name="o_pool", bufs=6) as o_pool,
        ):
            for rk in range(RK):
                # Stage the full [I, B] xT and [I, U] weight slices for this
                # position; contraction dim i = c*128 + p lands on partitions.
                xts = xt_pool.tile([P, IC, B], bf16)
                ws = w_pool.tile([P, IC, U], bf16)
                # Per-contraction-chunk DMAs so the first matmuls can start
                # before the whole 3MB slice has landed.
                for ic in range(IC):
                    nc.sync.dma_start(
                        out=xts[:, ic, :], in_=xt[rk, ic * P : (ic + 1) * P, :]
                    )
                    nc.sync.dma_start(
                        out=ws[:, ic, :], in_=w[rk, ic * P : (ic + 1) * P, :]
                    )

                for bcp in range(2):  # pairs of 128-row batch chunks
                    ps = psum_pool.tile([P, 4, 512], f32)  # 4 PSUM banks
                    for bc_i in range(2):
                        bc = bcp * 2 + bc_i
                        for ic in range(IC):
                            lhsT = xts[:, ic, bc * P : (bc + 1) * P]
                            for nck in range(NCK):
                                nc.tensor.matmul(
                                    ps[:, bc_i * NCK + nck, :],
                                    lhsT,
                                    ws[:, ic, nck * 512 : (nck + 1) * 512],
                                    start=(ic == 0),
                                    stop=(ic == IC - 1),
                                )
                    # softplus(z) = ln(exp(z) + 1); Exp in-place on PSUM,
                    # Ln evicts PSUM -> SBUF.  Both are one LUT table set.
                    nc.scalar.activation(
                        ps[:], ps[:], mybir.ActivationFunctionType.Exp
                    )
                    o = o_pool.tile([P, 4, 512], bf16)
                    nc.scalar.activation(
                        o[:], ps[:], mybir.ActivationFunctionType.Ln, bias=1.0
                    )
                    # Stores ride the SWDGE (GpSimd) so they never stall the
                    # ScalarE activation chain or the input ring.
                    for bc_i in range(2):
                        b0 = bcp * 256 + bc_i * P
                        nc.gpsimd.dma_start(
                            out=y[b0 : b0 + P, rk].rearrange(
                                "p (c n) -> p c n", c=NCK
                            ),
                            in_=o[:, bc_i * NCK : (bc_i + 1) * NCK, :],
                        )
    nc.compile()
    _dedupe_act_table_loads(nc)
    return nc


_NC_CACHE = None


def _get_nc():
    global _NC_CACHE
    if _NC_CACHE is None:
        _NC_CACHE = _build()
    return _NC_CACHE


def _prep_in_maps(x, kernels):
    in_maps = []
    for c in range(NCORES):
        ks = slice(c * RK, (c + 1) * RK)
        # [B, RK, I] -> [RK, I, B], contraction-major, bf16
        xtc = x[:, ks, :].transpose(1, 2, 0).astype(BF16)
        wc = np.ascontiguousarray(kernels[ks]).astype(BF16)
        in_maps.append({"xt": xtc, "w": wc})
    return in_maps


LAST_RESULT = None  # BassKernelResults of the most recent run (for test harness)


def kernel(x, kernels, _trace=False):
    global LAST_RESULT
    nc = _get_nc()
    in_maps = _prep_in_maps(np.asarray(x), np.asarray(kernels))
    res = run_bass_kernel_spmd(nc, in_maps, list(range(NCORES)), trace=_trace)
    LAST_RESULT = res
    out = np.concatenate([res.results[c]["y"] for c in range(NCORES)], axis=1)
    return out.astype(np.float32)


# revision 14
# speedup vs baseline: 1.0441x; 1.0097x over previous
"""Trainium2 Bass kernel for nn_DenseLocal: out = softplus(einsum('bki,kio->bko', x, kernels)).

Shapes (hardcoded): x [512, 128, 1024] f32, kernels [128, 1024, 1024] f32,
out [512, 128, 1024] f32.

Strategy: shard the 128 position-kernels across 8 NeuronCores (16 each,
expert-style).  Per core, each position k is an independent [512,1024] @
[1024,1024] GEMM followed by softplus.  Inputs are cast to bf16 on the host
(fp32 matmul is 4x slower on the PE; bf16 accumulates in fp32 PSUM), x is
pre-transposed on the host so the contraction dim lands on SBUF partitions.
Softplus is computed as Ln(Exp(z) + 1) on the ScalarE — both functions live
in one LUT table set; Softplus itself is not in this compiler's act tables.
"""

import sys
import types

import ml_dtypes
import numpy as np

BF16 = ml_dtypes.bfloat16

B = 512          # batch
K = 128          # n_kernels (position axis)
I = 1024         # in_dim
U = 1024         # units
NCORES = 8
RK = K // NCORES  # kernels per core
P = 128           # SBUF partitions
IC = I // P       # 8 contraction chunks
NCK = U // 512    # 2 moving chunks per units dim


def _ensure_axon_hooks():
    """The image's antenv package lacks axon_hooks; inject a minimal registry
    so run_bass_kernel_spmd(trace=True) can find the NTFF profile hook."""
    if "antenv.axon_hooks" in sys.modules:
        return
    hooks = types.ModuleType("antenv.axon_hooks")
    hooks._hook = None

    def _set(h):
        hooks._hook = h

    def _get():
        return hooks._hook

    hooks.set_axon_ntff_profile_hook = _set
    hooks.get_axon_ntff_profile_hook = _get
    try:
        import antenv

        sys.modules["antenv.axon_hooks"] = hooks
        antenv.axon_hooks = hooks
    except ImportError:
        pass


_ensure_axon_hooks()

import concourse.mybir as mybir  # noqa: E402
import concourse.tile as tile  # noqa: E402
from concourse import bacc  # noqa: E402
from concourse.bass_utils import run_bass_kernel_spmd  # noqa: E402
from concourse.hw_specs import get_activation_tables  # noqa: E402


def _dedupe_act_table_loads(nc):
    """bacc's insert_act_table_loads alternates exp_and_others /
    natural_log per activation (64 reloads x ~1.3us).  Both Exp and Ln
    live in the single natural_log_exp_and_others set: retarget the first
    load to it and drop the rest."""
    set_id = list(get_activation_tables(nc.m.arch)).index(
        "natural_log_exp_and_others"
    )
    first = True
    for blk in nc.main_func.blocks:
        drop = []
        for idx, inst in enumerate(blk.instructions):
            if isinstance(inst, mybir.InstLoadActFuncSet):
                assert inst.sync_info is None or (
                    not inst.sync_info.on_wait and not inst.sync_info.on_update
                )
                if first:
                    inst.act_func_set_id = set_id
                    first = False
                else:
                    drop.append(idx)
        for idx in reversed(drop):
            del blk.instructions[idx]


def _build():
    """Build the per-core Bass program.

    Per-core DRAM I/O:
      xt [RK, I, B]  bf16 — x shard, transposed per position (contraction-major)
      w  [RK, I, U]  bf16 — kernels shard, natural [in, out] layout
      y  [B, RK, U]  f32  — output shard
    """
    f32 = mybir.dt.float32
    bf16 = mybir.dt.bfloat16

    nc = bacc.Bacc()
    xt = nc.declare_dram_parameter("xt", [RK, I, B], bf16, isOutput=False)
    w = nc.declare_dram_parameter("w", [RK, I, U], bf16, isOutput=False)
    y = nc.declare_dram_parameter("y", [B, RK, U], bf16, isOutput=True)

    with tile.TileContext(nc) as tc:
        with (
            tc.tile_pool(name="xt_pool", bufs=4) as xt_pool,
            tc.tile_pool(name="w_pool", bufs=4) as w_pool,
            tc.tile_pool(name="psum_pool", bufs=4, space="PSUM") as psum_pool,
            tc.tile_pool(name="o_pool", bufs=8) as o_pool,
        ):
            for rk in range(RK):
                # Stage the full [I, B] xT and [I, U] weight slices for this
                # position; contraction dim i = c*128 + p lands on partitions.
                xts = xt_pool.tile([P, IC, B], bf16)
                ws = w_pool.tile([P, IC, U], bf16)
                # Per-contraction-chunk DMAs so the first matmuls can start
                # before the whole 3MB slice has landed.
                for ic in range(IC):
                    nc.sync.dma_start(
                        out=xts[:, ic, :], in_=xt[rk, ic * P : (ic + 1) * P, :]
                    )
                    nc.sync.dma_start(
                        out=ws[:, ic, :], in_=w[rk, ic * P : (ic + 1) * P, :]
                    )

                for bc in range(4):  # 128-row batch chunks
                    ps = psum_pool.tile([P, NCK, 512], f32)  # 2 PSUM banks
                    for ic in range(IC):
                        lhsT = xts[:, ic, bc * P : (bc + 1) * P]
                        for nck in range(NCK):
                            nc.tensor.matmul(
                                ps[:, nck, :],
                                lhsT,
                                ws[:, ic, nck * 512 : (nck + 1) * 512],
                                start=(ic == 0),
                                stop=(ic == IC - 1),
                            )
                    # softplus(z) = ln(exp(z) + 1); Exp in-place on PSUM,
                    # Ln evicts PSUM -> SBUF.  Both are one LUT table set.
                    nc.scalar.activation(
                        ps[:], ps[:], mybir.ActivationFunctionType.Exp
                    )
                    o = o_pool.tile([P, NCK, 512], bf16)
                    nc.scalar.activation(
                        o[:], ps[:], mybir.ActivationFunctionType.Ln, bias=1.0
                    )
                    # Stores ride the SWDGE (GpSimd) so they never stall the
                    # ScalarE activation chain or the input ring.
                    nc.gpsimd.dma_start(
                        out=y[bc * P : (bc + 1) * P, rk].rearrange(
                            "p (c n) -> p c n", c=NCK
                        ),
                        in_=o[:],
                    )
    nc.compile()
    _dedupe_act_table_loads(nc)
    return nc


_NC_CACHE = None


def _get_nc():
    global _NC_CACHE
    if _NC_CACHE is None:
        _NC_CACHE = _build()
    return _NC_CACHE


def _prep_in_maps(x, kernels):
    in_maps = []
    for c in range(NCORES):
        ks = slice(c * RK, (c + 1) * RK)
        # [B, RK, I] -> [RK, I, B], contraction-major, bf16
        xtc = x[:, ks, :].transpose(1, 2, 0).astype(BF16)
        wc = np.ascontiguousarray(kernels[ks]).astype(BF16)
        in_maps.append({"xt": xtc, "w": wc})
    return in_maps


LAST_RESULT = None  # BassKernelResults of the most recent run (for test harness)


def kernel(x, kernels, _trace=False):
    global LAST_RESULT
    nc = _get_nc()
    in_maps = _prep_in_maps(np.asarray(x), np.asarray(kernels))
    res = run_bass_kernel_spmd(nc, in_maps, list(range(NCORES)), trace=_trace)
    LAST_RESULT = res
    out = np.concatenate([res.results[c]["y"] for c in range(NCORES)], axis=1)
    return out.astype(np.float32)
